# revision 19
# baseline (speedup 1.0000x reference)
"""Trainium2 Bass kernel for CrossAttention (B=8, C=384, H=W=32, heads=6, dim_head=64).

Strategy: data-parallel over batch across 8 NeuronCores (1 batch element per
core, no collectives). Per core:
  - Q/K projections as [inner, N] (weights pre-transposed on host, qk scale
    folded into Wq), V projection produced directly transposed as vT [N, inner]
    by using the activation as the stationary matmul operand.
  - Per head: scoresT[j, i] = K_h^T Q_h via TensorE (fp32r), exp on ScalarE
    (PSUM -> SBUF bf16), then col-tiled TensorE matmuls compute both
    O_unnorm[dm, i] = vT^T expT and denom[i] = ones^T expT concurrently.
  - r = exp(-ln(denom)) on ScalarE (single activation table set), broadcast
    across partitions via DMA.
  - attn_map via fused DVE tensor_tensor_reduce: sum_i expT[j,i] * r[i],
    accumulated over heads.
  - Output projection with deferred per-token normalization (O * r) and the
    bias folded in as a rank-1 (K=1) accumulating matmul.
"""

import sys

sys.path.insert(0, "/opt/trn_rl_repo")

from contextlib import ExitStack

import numpy as np

import concourse.bass as bass
import concourse.bacc as bacc
import concourse.tile as tile
from concourse import library_config, mybir
from concourse.bass_utils import run_bass_kernel_spmd

B, C, HH, WW = 8, 384, 32, 32
N = HH * WW  # 1024 tokens
HEADS, DM = 6, 64
P = 128
NCH = C // P  # 3 channel chunks
NJC = N // P  # 8 j-chunks
SCALE = DM ** (-0.5)

F32 = mybir.dt.float32
F32R = mybir.dt.float32r
BF16 = mybir.dt.bfloat16
AF = mybir.ActivationFunctionType
ALU = mybir.AluOpType

SLOTS_PER_HEAD = 16  # (jc, ihalf): 8 * 2, each [128, 512] of scoresT
TOTAL_SLOTS = HEADS * SLOTS_PER_HEAD  # 96
SLOT_BLK = 3  # slots per scores psum tile / exp op


def r32(ap):
    return ap.bitcast(F32R)


def build_nc():
    nc = bacc.Bacc("TRN2", target_bir_lowering=False)

    xq_d = nc.dram_tensor("xq", [C, N], F32R, kind="ExternalInput")
    xc_d = nc.dram_tensor("xc", [C, N], F32R, kind="ExternalInput")
    wqt_d = nc.dram_tensor("wqt", [C, C], F32R, kind="ExternalInput")  # [c, inner], pre-scaled
    wkt_d = nc.dram_tensor("wkt", [C, C], F32R, kind="ExternalInput")  # [c, inner]
    wvt_d = nc.dram_tensor("wvt", [C, C], F32R, kind="ExternalInput")  # [c, inner]
    wot_d = nc.dram_tensor("wot", [C, C], F32R, kind="ExternalInput")  # [inner, c]
    bo_d = nc.dram_tensor("bo", [C], F32R, kind="ExternalInput")
    onesr_d = nc.dram_tensor("onesr", [1, 512], F32R, kind="ExternalInput")
    out_d = nc.dram_tensor("out", [C, N], F32, kind="ExternalOutput")
    am_d = nc.dram_tensor("am", [P, NJC], F32, kind="ExternalOutput")
    rsc_d = [nc.dram_tensor(f"rscratch{h}", [1, N], F32) for h in range(HEADS)]

    with tile.TileContext(nc) as tc:
        with ExitStack() as ctx:
            _body(ctx, tc, xq_d, xc_d, wqt_d, wkt_d, wvt_d, wot_d, bo_d, onesr_d, out_d, am_d, rsc_d)
    nc.compile()
    return nc


def _body(ctx, tc, xq_d, xc_d, wqt_d, wkt_d, wvt_d, wot_d, bo_d, onesr_d, out_d, am_d, rsc_d):
    nc = tc.nc

    wp = ctx.enter_context(tc.tile_pool(name="wp", bufs=1))
    big = ctx.enter_context(tc.tile_pool(name="big", bufs=1))
    rp = ctx.enter_context(tc.tile_pool(name="rp", bufs=2))
    scp = ctx.enter_context(tc.tile_pool(name="scp", bufs=2, space="PSUM"))
    avp = ctx.enter_context(tc.tile_pool(name="avp", bufs=1, space="PSUM"))

    # ---- persistent SBUF tiles ----
    wq_sb = [wp.tile([P, C], F32R, tag=f"wq{k}", name=f"wq{k}") for k in range(NCH)]
    wk_sb = [wp.tile([P, C], F32R, tag=f"wk{k}", name=f"wk{k}") for k in range(NCH)]
    wv_sb = [wp.tile([P, C], F32R, tag=f"wv{k}", name=f"wv{k}") for k in range(NCH)]
    wo_sb = [wp.tile([P, C], F32R, tag=f"wo{k}", name=f"wo{k}") for k in range(NCH)]
    bor = wp.tile([1, C], F32R, tag="bor")  # bias as a row on partition 0
    ones_row = wp.tile([1, 512], F32R, tag="ones_row")  # rhs for bias matmul
    ones_col = wp.tile([P, 1], BF16, tag="ones_col")  # lhsT for denom matmul
    am_a = wp.tile([P, NJC], F32, tag="am_a")
    am_b = wp.tile([P, NJC], F32, tag="am_b")

    q_sb = [big.tile([P, N], F32R, tag=f"q{m}", name=f"q{m}") for m in range(NCH)]
    k_sb = [big.tile([P, N], F32R, tag=f"k{m}", name=f"k{m}") for m in range(NCH)]
    vt_sb = [big.tile([P, C], BF16, tag=f"vt{j}", name=f"vt{j}") for j in range(NJC)]
    on_sb = [big.tile([P, N], F32R, tag=f"on{t}", name=f"on{t}") for t in range(NCH)]

    # ---- load constants / weights ----
    for k in range(NCH):
        nc.sync.dma_start(out=wq_sb[k], in_=wqt_d[k * P:(k + 1) * P, :])
        nc.sync.dma_start(out=wk_sb[k], in_=wkt_d[k * P:(k + 1) * P, :])
        nc.sync.dma_start(out=wv_sb[k], in_=wvt_d[k * P:(k + 1) * P, :])
        nc.sync.dma_start(out=wo_sb[k], in_=wot_d[k * P:(k + 1) * P, :])
    nc.sync.dma_start(out=bor, in_=bo_d[None, :])
    nc.sync.dma_start(out=ones_row, in_=onesr_d[:, :])
    nc.vector.memset(ones_col, 1.0)
    nc.vector.memset(am_a, 0.0)

    # ---- phase 1: projections (xq/xc in a scoped pool so SBUF is reclaimed) ----
    with tc.tile_pool(name="xp", bufs=1) as xp:
        xq_sb = [xp.tile([P, N], F32R, tag=f"xq{k}", name=f"xq{k}") for k in range(NCH)]
        xc_sb = [xp.tile([P, N], F32R, tag=f"xc{k}", name=f"xc{k}") for k in range(NCH)]
        for k in range(NCH):
            nc.sync.dma_start(out=xq_sb[k], in_=xq_d[k * P:(k + 1) * P, :])
            nc.sync.dma_start(out=xc_sb[k], in_=xc_d[k * P:(k + 1) * P, :])

        # Q = Wq^T-style: q[m*128:(m+1)*128, :] ; same for K. Emit m=0 first so
        # head 0 can start early.
        for m in range(NCH):
            for dst, w_t, x_t in ((q_sb, wq_sb, xq_sb), (k_sb, wk_sb, xc_sb)):
                for ih in range(2):
                    ps = scp.tile([P, 512], F32, tag="sc", name="ps")
                    for kc in range(NCH):
                        nc.tensor.matmul(
                            ps,
                            w_t[kc][:, m * P:(m + 1) * P],
                            x_t[kc][:, ih * 512:(ih + 1) * 512],
                            start=(kc == 0),
                            stop=(kc == NCH - 1),
                        )
                    nc.any.tensor_copy(dst[m][:, ih * 512:(ih + 1) * 512], ps)
            # vT chunks interleaved so they're ready before the first av
            if m == 0:
                for j in range(NJC):
                    ps = scp.tile([P, C], F32, tag="sc", name="psv")
                    for kc in range(NCH):
                        nc.tensor.matmul(
                            ps,
                            xc_sb[kc][:, j * P:(j + 1) * P],
                            wv_sb[kc][:, :],
                            start=(kc == 0),
                            stop=(kc == NCH - 1),
                        )
                    nc.any.tensor_copy(vt_sb[j], ps)

    # ---- phase 2: attention heads ----
    # expT per-head tiles (rotating pool): [j(128), jc*1024 + ih*512 + i]
    exp_tiles = {}
    av_tiles = {}

    def slot_mm(g):
        """qk matmul for global slot g into the current scores tile."""
        h, s = divmod(g, SLOTS_PER_HEAD)
        jc, ih = divmod(s, 2)
        hp = h % 2
        kt = k_sb[h // 2]
        qt = q_sb[h // 2]
        lhsT = kt[hp * DM:(hp + 1) * DM, jc * P:(jc + 1) * P]
        rhs = qt[hp * DM:(hp + 1) * DM, ih * 512:(ih + 1) * 512]
        return lhsT, rhs

    def emit_av(h, jc, ih):
        """col-tiled O_unnorm + denom accumulation for (h, jc, ih)."""
        if h not in av_tiles:
            av_tiles[h] = avp.tile([P, N], F32, tag="av", name=f"av{h}")
        avt = av_tiles[h]
        off = jc * 1024 + ih * 512
        rhs = exp_tiles[h][:, off:off + 512]
        even = (h % 2 == 0)
        o_lo, o_hi = (0, DM) if even else (DM, 2 * DM)
        o_pos = 0 if even else 64
        d_row = 64 if even else 0
        d_pos = 64 if even else 0
        nc.tensor.matmul(
            avt[o_lo:o_hi, ih * 512:(ih + 1) * 512],
            vt_sb[jc][:, h * DM:(h + 1) * DM],
            rhs,
            start=(jc == 0),
            stop=(jc == NJC - 1),
            tile_position=(0, o_pos),
            skip_group_check=True,
        )
        nc.tensor.matmul(
            avt[d_row:d_row + 1, ih * 512:(ih + 1) * 512],
            ones_col[:, 0:1],
            rhs,
            start=(jc == 0),
            stop=(jc == NJC - 1),
            tile_position=(0, d_pos),
            skip_group_check=True,
        )

    def finish_head(h):
        """denom -> r -> broadcast -> attn_map TTRs -> deferred norm."""
        avt = av_tiles[h]
        even = (h % 2 == 0)
        d_row = 64 if even else 0
        rows = slice(0, DM) if even else slice(DM, 2 * DM)

        lnt = rp.tile([P, N], F32, tag="lnrow", name=f"ln{h}")
        rrt = rp.tile([P, N], F32, tag="rrow", name=f"rr{h}")
        rb = rp.tile([P, N], F32, tag="rb", bufs=3, name=f"rb{h}")
        nc.scalar.activation(lnt[d_row:d_row + 1, :], avt[d_row:d_row + 1, :], AF.Ln)
        nc.scalar.activation(rrt[d_row:d_row + 1, :], lnt[d_row:d_row + 1, :], AF.Exp, scale=-1.0)
        nc.sync.dma_start(out=rsc_d[h][:, :], in_=rrt[d_row:d_row + 1, :])
        nc.sync.dma_start(out=rb, in_=rsc_d[h][0:1, :].to_broadcast([P, N]))

        # attn_map contribution: am[p, jc] += sum_i expT[p, jc*1024+i] * r[i] / 6144
        expt = exp_tiles[h]
        amp = rp.tile([P, NJC], F32, tag="amp", name=f"amp{h}")
        for jc in range(NJC):
            off = jc * 1024
            stto = rp.tile([P, N], BF16, tag="stto", name=f"stto{h}_{jc}")
            nc.vector.scalar_tensor_tensor(
                out=stto,
                in0=expt[:, off:off + N],
                scalar=1.0 / (HEADS * N),
                in1=rb,
                op0=ALU.mult,
                op1=ALU.mult,
                accum_out=amp[:, jc:jc + 1],
            )
        src, dst = (am_a, am_b) if h % 2 == 0 else (am_b, am_a)
        nc.vector.tensor_tensor(out=dst, in0=src, in1=amp, op=ALU.add)

        # deferred softmax normalization of O_unnorm
        pair = h // 2
        o_lo, o_hi = (0, DM) if even else (DM, 2 * DM)
        nc.vector.tensor_tensor(
            out=on_sb[pair][o_lo:o_hi, :],
            in0=avt[o_lo:o_hi, :],
            in1=rb[o_lo:o_hi, :],
            op=ALU.mult,
        )
        del av_tiles[h]

    # interleaved emission: per head, qk -> exp (blocks of up to 3 slots)
    # -> av/denom -> head tail
    for h in range(HEADS):
        exp_tiles[h] = big.tile(
            [P, SLOTS_PER_HEAD * 512], BF16, tag="expt", bufs=3, name=f"expt{h}"
        )
        s0 = 0
        while s0 < SLOTS_PER_HEAD:
            nblk = min(SLOT_BLK, SLOTS_PER_HEAD - s0)
            sct = scp.tile([P, nblk * 512], F32, tag="sc", name=f"sc{h}_{s0}")
            for t in range(nblk):
                lhsT, rhs = slot_mm(h * SLOTS_PER_HEAD + s0 + t)
                nc.tensor.matmul(
                    sct[:, t * 512:(t + 1) * 512],
                    lhsT,
                    rhs,
                    start=True,
                    stop=True,
                )
            nc.scalar.activation(
                exp_tiles[h][:, s0 * 512:(s0 + nblk) * 512],
                sct[:, 0:nblk * 512],
                AF.Exp,
            )
            for t in range(nblk):
                jc, ih = divmod(s0 + t, 2)
                emit_av(h, jc, ih)
            s0 += nblk
        finish_head(h)
        if h >= 2:
            del exp_tiles[h - 2]

    # ---- phase 3: output projection (bias folded as K=1 matmul) ----
    for cc in range(NCH):
        for ih in range(2):
            ps = scp.tile([P, 512], F32, tag="sc", name="ps")
            for kc in range(NCH):
                nc.tensor.matmul(
                    ps,
                    wo_sb[kc][:, cc * P:(cc + 1) * P],
                    on_sb[kc][:, ih * 512:(ih + 1) * 512],
                    start=(kc == 0),
                    stop=False,
                    skip_group_check=True,
                )
            nc.tensor.matmul(
                ps,
                bor[0:1, cc * P:(cc + 1) * P],
                ones_row[0:1, :],
                start=False,
                stop=True,
                skip_group_check=True,
            )
            ot = rp.tile([P, 512], F32, tag="ot", name=f"ot{cc}_{ih}")
            nc.any.tensor_copy(ot, ps)
            nc.sync.dma_start(
                out=out_d[cc * P:(cc + 1) * P, ih * 512:(ih + 1) * 512], in_=ot
            )

    nc.sync.dma_start(out=am_d[:, :], in_=am_a)


_NC_CACHE = None
_RUN_KWARGS = {}
_LAST_PROFILE = {}


def _get_nc():
    global _NC_CACHE
    if _NC_CACHE is None:
        _NC_CACHE = build_nc()
    return _NC_CACHE


def kernel(query, context, Wq, Wk, Wv, Wo, bo):
    query = np.asarray(query, dtype=np.float32)
    context = np.asarray(context, dtype=np.float32)
    wqt = (np.asarray(Wq, dtype=np.float32).T * SCALE).copy()
    wkt = np.asarray(Wk, dtype=np.float32).T.copy()
    wvt = np.asarray(Wv, dtype=np.float32).T.copy()
    wot = np.asarray(Wo, dtype=np.float32).T.copy()
    bo = np.asarray(bo, dtype=np.float32)

    nc = _get_nc()
    in_maps = []
    for b in range(B):
        in_maps.append({
            "xq": query[b].reshape(C, N).copy(),
            "xc": context[b].reshape(C, N).copy(),
            "wqt": wqt,
            "wkt": wkt,
            "wvt": wvt,
            "wot": wot,
            "bo": bo,
            "onesr": np.ones((1, 512), dtype=np.float32),
        })
    res = run_bass_kernel_spmd(nc, in_maps, core_ids=list(range(B)), **_RUN_KWARGS)
    if res.exec_time_ns is not None:
        print(f"HW exec time: {res.exec_time_ns} ns")
        _LAST_PROFILE.clear()
        _LAST_PROFILE.update(exec_time_ns=res.exec_time_ns, profile_json=res.profile_json)
    outs = np.stack([res.results[b]["out"].reshape(C, HH, WW) for b in range(B)])
    ams = np.stack(
        [res.results[b]["am"].T.reshape(HH, WW) for b in range(B)]
    )
    return outs.astype(np.float32), ams.astype(np.float32)


# revision 21
# speedup vs baseline: 1.1628x; 1.1628x over previous
"""Trainium2 Bass kernel for CrossAttention (B=8, C=384, H=W=32, heads=6, dim_head=64).

Strategy: data-parallel over batch across 8 NeuronCores (1 batch element per
core, no collectives). Per core:
  - Q/K projections as [inner, N] (weights pre-transposed on host, qk scale
    folded into Wq), V projection produced directly transposed as vT [N, inner]
    by using the activation as the stationary matmul operand.
  - Per head: scoresT[j, i] = K_h^T Q_h via TensorE (fp32r), exp on ScalarE
    (PSUM -> SBUF bf16), then col-tiled TensorE matmuls compute both
    O_unnorm[dm, i] = vT^T expT and denom[i] = ones^T expT concurrently.
  - r = exp(-ln(denom)) on ScalarE (single activation table set), broadcast
    across partitions via DMA.
  - attn_map via fused DVE tensor_tensor_reduce: sum_i expT[j,i] * r[i],
    accumulated over heads.
  - Output projection with deferred per-token normalization (O * r) and the
    bias folded in as a rank-1 (K=1) accumulating matmul.
"""

import sys

sys.path.insert(0, "/opt/trn_rl_repo")

from contextlib import ExitStack

import numpy as np

import concourse.bass as bass
import concourse.bacc as bacc
import concourse.tile as tile
from concourse import library_config, mybir
from concourse.bass_utils import run_bass_kernel_spmd

B, C, HH, WW = 8, 384, 32, 32
N = HH * WW  # 1024 tokens
HEADS, DM = 6, 64
P = 128
NCH = C // P  # 3 channel chunks
NJC = N // P  # 8 j-chunks
SCALE = DM ** (-0.5)

F32 = mybir.dt.float32
F32R = mybir.dt.float32r
BF16 = mybir.dt.bfloat16
AF = mybir.ActivationFunctionType
ALU = mybir.AluOpType

SLOTS_PER_HEAD = 16  # (jc, ihalf): 8 * 2, each [128, 512] of scoresT
TOTAL_SLOTS = HEADS * SLOTS_PER_HEAD  # 96
SLOT_BLK = 3  # slots per scores psum tile / exp op


def r32(ap):
    return ap.bitcast(F32R)


class _Bacc(bacc.Bacc):
    """Bacc that resolves every activation to one table set.

    Exp/Ln/Copy/Identity all live in natural_log_exp_and_others; stripping
    them from every other set makes the table-load pass emit a single
    ACT_TABLE_LOAD instead of thrashing between exp/ln sets per head.
    """

    def insert_act_table_loads(self):
        import bass_rust as _br
        from concourse.hw_specs import get_activation_tables

        keep = {"Exp", "Ln", "Copy", "Identity"}
        tables = []
        for name, funcs in get_activation_tables(self.m.arch).items():
            if name != "natural_log_exp_and_others":
                funcs = {f for f in funcs if f.name not in keep}
            tables.append((name, funcs))
        _br.insert_act_table_loads(self, tables)


def build_nc():
    nc = _Bacc("TRN2", target_bir_lowering=False)

    xq_d = nc.dram_tensor("xq", [C, N], F32R, kind="ExternalInput")
    xc_d = nc.dram_tensor("xc", [C, N], F32R, kind="ExternalInput")
    wqt_d = nc.dram_tensor("wqt", [C, C], F32R, kind="ExternalInput")  # [c, inner], pre-scaled
    wkt_d = nc.dram_tensor("wkt", [C, C], F32R, kind="ExternalInput")  # [c, inner]
    wvt_d = nc.dram_tensor("wvt", [C, C], F32R, kind="ExternalInput")  # [c, inner]
    wot_d = nc.dram_tensor("wot", [C, C], F32R, kind="ExternalInput")  # [inner, c]
    bo_d = nc.dram_tensor("bo", [C], F32R, kind="ExternalInput")
    onesr_d = nc.dram_tensor("onesr", [1, 512], F32R, kind="ExternalInput")
    out_d = nc.dram_tensor("out", [C, N], F32, kind="ExternalOutput")
    am_d = nc.dram_tensor("am", [P, NJC], F32, kind="ExternalOutput")
    rsc_d = [nc.dram_tensor(f"rscratch{h}", [1, N], F32) for h in range(HEADS)]

    with tile.TileContext(nc) as tc:
        with ExitStack() as ctx:
            _body(ctx, tc, xq_d, xc_d, wqt_d, wkt_d, wvt_d, wot_d, bo_d, onesr_d, out_d, am_d, rsc_d)
    nc.compile()
    return nc


def _body(ctx, tc, xq_d, xc_d, wqt_d, wkt_d, wvt_d, wot_d, bo_d, onesr_d, out_d, am_d, rsc_d):
    nc = tc.nc

    wp = ctx.enter_context(tc.tile_pool(name="wp", bufs=1))
    big = ctx.enter_context(tc.tile_pool(name="big", bufs=1))
    rp = ctx.enter_context(tc.tile_pool(name="rp", bufs=2))
    scp = ctx.enter_context(tc.tile_pool(name="scp", bufs=2, space="PSUM"))
    avp = ctx.enter_context(tc.tile_pool(name="avp", bufs=2, space="PSUM"))

    # ---- persistent SBUF tiles ----
    wq_sb = [wp.tile([P, C], F32R, tag=f"wq{k}", name=f"wq{k}") for k in range(NCH)]
    wk_sb = [wp.tile([P, C], F32R, tag=f"wk{k}", name=f"wk{k}") for k in range(NCH)]
    wv_sb = [wp.tile([P, C], F32R, tag=f"wv{k}", name=f"wv{k}") for k in range(NCH)]
    wo_sb = [wp.tile([P, C], F32R, tag=f"wo{k}", name=f"wo{k}") for k in range(NCH)]
    bor = wp.tile([1, C], F32R, tag="bor")  # bias as a row on partition 0
    ones_row = wp.tile([1, 512], F32R, tag="ones_row")  # rhs for bias matmul
    ones_col = wp.tile([P, 1], BF16, tag="ones_col")  # lhsT for denom matmul
    am_a = wp.tile([P, NJC], F32, tag="am_a")
    am_b = wp.tile([P, NJC], F32, tag="am_b")

    q_sb = [big.tile([P, N], F32R, tag=f"q{m}", name=f"q{m}") for m in range(NCH)]
    k_sb = [big.tile([P, N], F32R, tag=f"k{m}", name=f"k{m}") for m in range(NCH)]
    vt_sb = [big.tile([P, C], BF16, tag=f"vt{j}", name=f"vt{j}") for j in range(NJC)]
    on_sb = [big.tile([P, N], F32R, tag=f"on{t}", name=f"on{t}") for t in range(NCH)]

    # ---- load constants / weights ----
    for k in range(NCH):
        nc.sync.dma_start(out=wq_sb[k], in_=wqt_d[k * P:(k + 1) * P, :])
        nc.sync.dma_start(out=wk_sb[k], in_=wkt_d[k * P:(k + 1) * P, :])
        nc.sync.dma_start(out=wv_sb[k], in_=wvt_d[k * P:(k + 1) * P, :])
        nc.sync.dma_start(out=wo_sb[k], in_=wot_d[k * P:(k + 1) * P, :])
    nc.sync.dma_start(out=bor, in_=bo_d[None, :])
    nc.sync.dma_start(out=ones_row, in_=onesr_d[:, :])
    nc.vector.memset(ones_col, 1.0)
    nc.vector.memset(am_a, 0.0)

    # ---- phase 1: projections (xq/xc in a scoped pool so SBUF is reclaimed) ----
    with tc.tile_pool(name="xp", bufs=1) as xp:
        xq_sb = [xp.tile([P, N], F32R, tag=f"xq{k}", name=f"xq{k}") for k in range(NCH)]
        xc_sb = [xp.tile([P, N], F32R, tag=f"xc{k}", name=f"xc{k}") for k in range(NCH)]
        for k in range(NCH):
            nc.sync.dma_start(out=xq_sb[k], in_=xq_d[k * P:(k + 1) * P, :])
            nc.sync.dma_start(out=xc_sb[k], in_=xc_d[k * P:(k + 1) * P, :])

        # Q = Wq^T-style: q[m*128:(m+1)*128, :] ; same for K. Emit m=0 first so
        # head 0 can start early.
        for m in range(NCH):
            for dst, w_t, x_t in ((q_sb, wq_sb, xq_sb), (k_sb, wk_sb, xc_sb)):
                for ih in range(2):
                    ps = scp.tile([P, 512], F32, tag="sc", name="ps")
                    for kc in range(NCH):
                        nc.tensor.matmul(
                            ps,
                            w_t[kc][:, m * P:(m + 1) * P],
                            x_t[kc][:, ih * 512:(ih + 1) * 512],
                            start=(kc == 0),
                            stop=(kc == NCH - 1),
                        )
                    nc.any.tensor_copy(dst[m][:, ih * 512:(ih + 1) * 512], ps)
            # vT chunks interleaved so they're ready before the first av
            if m == 0:
                for j in range(NJC):
                    ps = scp.tile([P, C], F32, tag="sc", name="psv")
                    for kc in range(NCH):
                        nc.tensor.matmul(
                            ps,
                            xc_sb[kc][:, j * P:(j + 1) * P],
                            wv_sb[kc][:, :],
                            start=(kc == 0),
                            stop=(kc == NCH - 1),
                        )
                    nc.any.tensor_copy(vt_sb[j], ps)

    # ---- phase 2: attention heads ----
    # expT per-head tiles (rotating pool): [j(128), jc*1024 + ih*512 + i]
    exp_tiles = {}
    av_tiles = {}

    def slot_mm(g):
        """qk matmul for global slot g into the current scores tile."""
        h, s = divmod(g, SLOTS_PER_HEAD)
        jc, ih = divmod(s, 2)
        hp = h % 2
        kt = k_sb[h // 2]
        qt = q_sb[h // 2]
        lhsT = kt[hp * DM:(hp + 1) * DM, jc * P:(jc + 1) * P]
        rhs = qt[hp * DM:(hp + 1) * DM, ih * 512:(ih + 1) * 512]
        return lhsT, rhs

    def emit_av(h, jc, ih):
        """col-tiled O_unnorm + denom accumulation for (h, jc, ih)."""
        if h not in av_tiles:
            av_tiles[h] = avp.tile([P, N], F32, tag="av", name=f"av{h}")
        avt = av_tiles[h]
        off = jc * 1024 + ih * 512
        rhs = exp_tiles[h][:, off:off + 512]
        even = (h % 2 == 0)
        o_lo, o_hi = (0, DM) if even else (DM, 2 * DM)
        o_pos = 0 if even else 64
        d_row = 64 if even else 0
        d_pos = 64 if even else 0
        nc.tensor.matmul(
            avt[o_lo:o_hi, ih * 512:(ih + 1) * 512],
            vt_sb[jc][:, h * DM:(h + 1) * DM],
            rhs,
            start=(jc == 0),
            stop=(jc == NJC - 1),
            tile_position=(0, o_pos),
            skip_group_check=True,
        )
        nc.tensor.matmul(
            avt[d_row:d_row + 1, ih * 512:(ih + 1) * 512],
            ones_col[:, 0:1],
            rhs,
            start=(jc == 0),
            stop=(jc == NJC - 1),
            tile_position=(0, d_pos),
            skip_group_check=True,
        )

    def finish_head(h):
        """denom -> r -> broadcast -> attn_map TTRs -> deferred norm."""
        avt = av_tiles[h]
        even = (h % 2 == 0)
        d_row = 64 if even else 0
        rows = slice(0, DM) if even else slice(DM, 2 * DM)

        lnt = rp.tile([P, N], F32, tag="lnrow", name=f"ln{h}")
        rrt = rp.tile([P, N], F32, tag="rrow", name=f"rr{h}")
        rb = rp.tile([P, N], F32, tag="rb", bufs=3, name=f"rb{h}")
        nc.scalar.activation(lnt[d_row:d_row + 1, :], avt[d_row:d_row + 1, :], AF.Ln)
        nc.scalar.activation(rrt[d_row:d_row + 1, :], lnt[d_row:d_row + 1, :], AF.Exp, scale=-1.0)
        nc.sync.dma_start(out=rsc_d[h][:, :], in_=rrt[d_row:d_row + 1, :])
        nc.sync.dma_start(out=rb, in_=rsc_d[h][0:1, :].to_broadcast([P, N]))

        # deferred softmax normalization of O_unnorm (before the am STTs so
        # the av psum tile is released promptly)
        o_lo, o_hi = (0, DM) if even else (DM, 2 * DM)
        pair = h // 2
        nc.vector.tensor_tensor(
            out=on_sb[pair][o_lo:o_hi, :],
            in0=avt[o_lo:o_hi, :],
            in1=rb[o_lo:o_hi, :],
            op=ALU.mult,
        )

        # attn_map contribution: am[p, jc] += sum_i expT[p, jc*1024+i] * r[i] / 6144
        expt = exp_tiles[h]
        amp = rp.tile([P, NJC], F32, tag="amp", name=f"amp{h}")
        for jc in range(NJC):
            off = jc * 1024
            stto = rp.tile([P, N], BF16, tag="stto", name=f"stto{h}_{jc}")
            nc.vector.scalar_tensor_tensor(
                out=stto,
                in0=expt[:, off:off + N],
                scalar=1.0 / (HEADS * N),
                in1=rb,
                op0=ALU.mult,
                op1=ALU.mult,
                accum_out=amp[:, jc:jc + 1],
            )
        src, dst = (am_a, am_b) if h % 2 == 0 else (am_b, am_a)
        nc.vector.tensor_tensor(out=dst, in0=src, in1=amp, op=ALU.add)
        del av_tiles[h]

    # interleaved emission: per head, per j-chunk: qk (2 MMs) -> exp -> av
    for h in range(HEADS):
        exp_tiles[h] = big.tile(
            [P, SLOTS_PER_HEAD * 512], BF16, tag="expt", bufs=3, name=f"expt{h}"
        )
        for jc in range(NJC):
            sct = scp.tile([P, N], F32, tag="sc", name=f"sc{h}_{jc}")
            for ih in range(2):
                lhsT, rhs = slot_mm(h * SLOTS_PER_HEAD + jc * 2 + ih)
                nc.tensor.matmul(
                    sct[:, ih * 512:(ih + 1) * 512],
                    lhsT,
                    rhs,
                    start=True,
                    stop=True,
                )
            nc.scalar.activation(
                exp_tiles[h][:, jc * 1024:(jc + 1) * 1024], sct, AF.Exp
            )
            for ih in range(2):
                emit_av(h, jc, ih)
        finish_head(h)
        if h >= 2:
            del exp_tiles[h - 2]

    # ---- phase 3: output projection (bias folded as K=1 matmul) ----
    for cc in range(NCH):
        for ih in range(2):
            ps = scp.tile([P, 512], F32, tag="sc", name="ps")
            for kc in range(NCH):
                nc.tensor.matmul(
                    ps,
                    wo_sb[kc][:, cc * P:(cc + 1) * P],
                    on_sb[kc][:, ih * 512:(ih + 1) * 512],
                    start=(kc == 0),
                    stop=False,
                    skip_group_check=True,
                )
            nc.tensor.matmul(
                ps,
                bor[0:1, cc * P:(cc + 1) * P],
                ones_row[0:1, :],
                start=False,
                stop=True,
                skip_group_check=True,
            )
            ot = rp.tile([P, 512], F32, tag="ot", name=f"ot{cc}_{ih}")
            nc.any.tensor_copy(ot, ps)
            nc.sync.dma_start(
                out=out_d[cc * P:(cc + 1) * P, ih * 512:(ih + 1) * 512], in_=ot
            )

    nc.sync.dma_start(out=am_d[:, :], in_=am_a)


_NC_CACHE = None
_RUN_KWARGS = {}
_LAST_PROFILE = {}


def _get_nc():
    global _NC_CACHE
    if _NC_CACHE is None:
        _NC_CACHE = build_nc()
    return _NC_CACHE


def kernel(query, context, Wq, Wk, Wv, Wo, bo):
    query = np.asarray(query, dtype=np.float32)
    context = np.asarray(context, dtype=np.float32)
    wqt = (np.asarray(Wq, dtype=np.float32).T * SCALE).copy()
    wkt = np.asarray(Wk, dtype=np.float32).T.copy()
    wvt = np.asarray(Wv, dtype=np.float32).T.copy()
    wot = np.asarray(Wo, dtype=np.float32).T.copy()
    bo = np.asarray(bo, dtype=np.float32)

    nc = _get_nc()
    in_maps = []
    for b in range(B):
        in_maps.append({
            "xq": query[b].reshape(C, N).copy(),
            "xc": context[b].reshape(C, N).copy(),
            "wqt": wqt,
            "wkt": wkt,
            "wvt": wvt,
            "wot": wot,
            "bo": bo,
            "onesr": np.ones((1, 512), dtype=np.float32),
        })
    res = run_bass_kernel_spmd(nc, in_maps, core_ids=list(range(B)), **_RUN_KWARGS)
    if res.exec_time_ns is not None:
        print(f"HW exec time: {res.exec_time_ns} ns")
        _LAST_PROFILE.clear()
        _LAST_PROFILE.update(exec_time_ns=res.exec_time_ns, profile_json=res.profile_json)
    outs = np.stack([res.results[b]["out"].reshape(C, HH, WW) for b in range(B)])
    ams = np.stack(
        [res.results[b]["am"].T.reshape(HH, WW) for b in range(B)]
    )
    return outs.astype(np.float32), ams.astype(np.float32)
